# revision 22
# baseline (speedup 1.0000x reference)
"""MoE layer (top-2 of 8 experts, H=1024, FFN=4096) on 8 TRN2 NeuronCores.

Expert-parallel: core e holds expert e's weights. The (tiny) router runs on
host; tokens are gathered per-expert into capacity-padded batches, each core
runs the expert FFN and the host applies gate * (y + b2) while scatter-adding
the two expert contributions per token.

Device layout per core (C = per-expert token capacity, multiple of 8, split
into c-blocks of <=512 tokens; all weights SBUF-resident, c-block outer loop):
  GEMM1  h[f, c]  = w1[k, f-tile].T @ x[k, c]   (F-tile on psum partitions,
         tokens free; contraction over 8 k-tiles of H)
  GEMM2  y[n, c]  = w2[f, n-tile].T @ h[f, c]   (H-tile on psum partitions,
         tokens free; single 32-matmul PSUM accumulation over all of F)
Tokens are the moving operand in BOTH gemms, so PE cost scales with C rather
than with 128-rounded token tiles, and each block's outputs stream to HBM
as soon as its GEMM2 n-tile finishes (no output tail).  b1 is the gelu
activation bias (per-partition), b2 and the router gate are folded on host
into the final scatter-add; the kernel returns y^T = (x@w1.T->gelu)@w2.T as
[H, C] per core.

GEMMs run in bf16 (fp32 matmul on this PE is 2 passes at half clock = 4x
slower; fp16 hangs the exec unit; fp8 DoubleRow fails the 2e-2 error gate:
measured 3.7-5.2e-2). PSUM accumulation, bias, gelu stay fp32; end-to-end
absmax-relative error vs the fp32 reference is ~3.5e-3.
"""

import os

os.environ.setdefault("NEURON_RT_RESET_CORES", "1")

import ml_dtypes
import numpy as np

import concourse.bass as bass  # noqa: F401  (bass types via bacc)
import concourse.mybir as mybir
from concourse import bacc
from concourse.tile import TileContext
from concourse.bass_utils import run_bass_kernel_spmd

H = 1024
E = 8
F = 4096
TOPK = 2
P = 128
N_CORES = 8
KH = H // P          # 8 contraction k-tiles for GEMM1
MI = F // P          # 32 F m-tiles (GEMM1 outputs / GEMM2 contraction)
NT = H // P          # 8 output n-tiles for GEMM2
FP32 = mybir.dt.float32
BF16 = mybir.dt.bfloat16

_cache: dict = {}

# Test-harness knobs (harness-safe defaults): set TRACE=True before calling
# kernel() to profile the device run; exec time lands in LAST_EXEC_TIME_NS.
TRACE = False
LAST_EXEC_TIME_NS = None


def _blocks(C: int):
    """Near-even c-block widths <=512, all multiples of 8.

    Tokens are the moving operand everywhere, so block width only affects
    per-matmul efficiency: keep every block >=~256 wide (a narrow block
    makes its matmuls LDWEIGHTS-bound).
    """
    assert C % 8 == 0
    nb = -(-C // 512)
    if nb >= 2 and 256 * (nb - 1) <= C - 400 <= 512 * (nb - 1):
        # First block slightly wide (400): its GEMM1 m-tiles then consume w1
        # slabs a touch slower than the startup DMA ramp delivers them, so
        # the early weight-arrival race never stalls the PE.
        widths = [400]
        rest, n_rest = C - 400, nb - 1
    else:
        widths = []
        rest, n_rest = C, nb
    base = (rest // n_rest) // 8 * 8
    widths += [base] * n_rest
    rem = rest - base * n_rest
    assert rem % 8 == 0 and rem // 8 <= n_rest
    for i in range(rem // 8):
        widths[len(widths) - n_rest + i] += 8
    assert all(0 < w <= 512 for w in widths) and sum(widths) == C
    cbs = []
    off = 0
    for w in widths:
        cbs.append((off, w))
        off += w
    return cbs


def _build(C: int):
    """Build + compile the per-core expert-FFN program for capacity C."""
    cbs = _blocks(C)

    nc = bacc.Bacc("TRN2", target_bir_lowering=False, debug=False,
                   num_devices=N_CORES)

    # Host-prepared layouts (see kernel()): all DMAs are contiguous
    # per-partition runs.
    xd = nc.dram_tensor("xd", [P, KH * C], BF16, kind="ExternalInput")
    w1d = nc.dram_tensor("w1d", [P, NT * KH * 512], BF16, kind="ExternalInput")
    w2d = nc.dram_tensor("w2d", [P, NT * MI * P], BF16, kind="ExternalInput")
    b1d = nc.dram_tensor("b1d", [P, MI], FP32, kind="ExternalInput")
    outd = nc.dram_tensor("out", [P, NT * C], FP32, kind="ExternalOutput")

    w1v = w1d.rearrange("p (a f) -> p a f", f=512)    # [128, 64, 512] (a=th*8+k)
    w2v = w2d.rearrange("p (n q) -> p n q", q=MI * P)  # [128, 8, 4096]

    GELU = mybir.ActivationFunctionType.Gelu

    with TileContext(nc) as tc:
        with (
            tc.tile_pool(name="cst", bufs=1) as cst,
            tc.tile_pool(name="xp", bufs=1) as xp,
            tc.tile_pool(name="w1m", bufs=1) as w1m,
            tc.tile_pool(name="w1p", bufs=1) as w1p,
            tc.tile_pool(name="w2p", bufs=1) as w2p,
            tc.tile_pool(name="hp", bufs=2) as hp,
            tc.tile_pool(name="op", bufs=3) as op,
            tc.tile_pool(name="ps1", bufs=3, space="PSUM") as ps1,
            tc.tile_pool(name="ps2", bufs=3, space="PSUM") as ps2,
            tc.tile_pool(name="psw", bufs=1, space="PSUM") as psw,
        ):
            # DMA emission order = arrival order, and each DMA instruction
            # costs ~0.7us of issue time on the sync queue.  Startup critical
            # path is x(block 0) + the first th0 m-tile of w1 (th0 is split
            # into 4 m-granular tiles so GEMM1 starts after ~1MB instead of
            # ~1.8MB); b1 is tiny and only needed at the first gelu, so it
            # issues after the critical pair.
            x_sb = {}

            def load_x(b):
                coff, cw = cbs[b]
                t = xp.tile([P, KH * cw], BF16, tag=f"x{b}", name=f"x{b}")
                nc.sync.dma_start(out=t[:], in_=xd[:, KH * coff:KH * (coff + cw)])
                x_sb[b] = t

            def x_ap(b, k, cw):
                return x_sb[b][:, k * cw:(k + 1) * cw]

            load_x(0)
            # w1 thunk 0, m-granular: 4 tiles of [128, 8k*128f].  The th0
            # region of w1d is laid out m-major on host so each m-tile DMA is
            # one contiguous 2KB/partition run (a k-strided slice would move
            # 256B chunks and crawl at descriptor-overhead rates).
            w1_th0 = []
            for m in range(4):
                t = w1m.tile([P, KH * P], BF16, tag=f"w1t0m{m}", name=f"w1_0_{m}")
                nc.sync.dma_start(out=t[:], in_=w1d[:, m * KH * P:(m + 1) * KH * P])
                w1_th0.append(t)
            # w1 thunks 1..7, slab-granular: [128, 8k, 512f].  b1 (tiny, first
            # needed at the first gelu ~2us after th1) issues after th1 so it
            # does not cost th1 an issue slot on the startup critical path.
            w1_sb = [None]
            t = w1p.tile([P, KH, 512], BF16, tag="w1_1", name="w1_1")
            nc.sync.dma_start(out=t[:], in_=w1v[:, KH:2 * KH, :])
            w1_sb.append(t)
            b1_sb = cst.tile([P, MI], FP32, tag="b1")
            nc.sync.dma_start(out=b1_sb[:], in_=b1d[:])
            for th in range(2, NT):
                t = w1p.tile([P, KH, 512], BF16, tag=f"w1_{th}", name=f"w1_{th}")
                nc.sync.dma_start(out=t[:], in_=w1v[:, th * KH:(th + 1) * KH, :])
                w1_sb.append(t)
            # w2, n-granular: [128, 32m*128n] per output n-tile
            w2_sb = []
            for n in range(NT):
                t = w2p.tile([P, MI * P], BF16, tag=f"w2_{n}", name=f"w2_{n}")
                nc.sync.dma_start(out=t[:], in_=w2v[:, n, :])
                w2_sb.append(t)
            for b in range(1, len(cbs)):
                load_x(b)

            # PE warm-up: ~32 junk matmuls on a memset tile run during the
            # startup DMA window (no data deps), so the HAM clock gate is at
            # K=8/8 before the first real matmul instead of ~3us after it.
            wtile = cst.tile([P, P], BF16, tag="warm")
            nc.vector.memset(wtile[:], 0.0)
            for _ in range(4):
                pw = psw.tile([P, P], FP32, tag="pw")
                for i in range(8):
                    nc.tensor.matmul(pw[:], wtile[:], wtile[:],
                                     start=(i == 0), stop=(i == 7))

            for b, (coff, cw) in enumerate(cbs):
                # h split in two tiles so GEMM2's first half only waits on
                # gelu(m=15), not on the final gelu of the block
                h_t = [hp.tile([P, (MI // 2) * cw], BF16, tag=f"h{hh}",
                               name=f"h{hh}_{b}")
                       for hh in range(2)]
                for th in range(NT):
                    for m in range(4):
                        mi = th * 4 + m
                        pt = ps1.tile([P, cw], FP32, tag="ps1")
                        for k in range(KH):
                            if th == 0:
                                w_ap = w1_th0[m][:, k * P:(k + 1) * P]
                            else:
                                w_ap = w1_sb[th][:, k, m * P:(m + 1) * P]
                            nc.tensor.matmul(
                                pt[:], w_ap,
                                x_ap(b, k, cw),
                                start=(k == 0), stop=(k == KH - 1),
                            )
                        mj = mi % (MI // 2)
                        nc.scalar.activation(
                            h_t[mi // (MI // 2)][:, mj * cw:(mj + 1) * cw],
                            pt[:], GELU, bias=b1_sb[:, mi:mi + 1],
                        )
                for n in range(NT):
                    pt2 = ps2.tile([P, cw], FP32, tag="ps2")
                    for m in range(MI):
                        mj = m % (MI // 2)
                        nc.tensor.matmul(
                            pt2[:], w2_sb[n][:, m * P:(m + 1) * P],
                            h_t[m // (MI // 2)][:, mj * cw:(mj + 1) * cw],
                            start=(m == 0), stop=(m == MI - 1),
                        )
                    o_t = op.tile([P, cw], FP32, tag="o")
                    nc.vector.tensor_scalar_add(o_t[:], pt2[:], 0.0)
                    nc.sync.dma_start(out=outd[:, n * C + coff:n * C + coff + cw],
                                      in_=o_t[:])

    nc.compile()
    return nc


def _route(x: np.ndarray, router_w: np.ndarray):
    """Host router: top-2 expert ids + softmax gates per token."""
    logits = x @ router_w.T                                   # [T, E]
    top_i = np.argsort(-logits, axis=1, kind="stable")[:, :TOPK]
    top_v = np.take_along_axis(logits, top_i, axis=1)
    mx = top_v.max(axis=1, keepdims=True)
    ex = np.exp(top_v - mx)
    rw = ex / ex.sum(axis=1, keepdims=True)
    return top_i, rw.astype(np.float32)


def kernel(hidden_states, router_w, w1, b1, w2, b2):
    hidden_states = np.ascontiguousarray(np.asarray(hidden_states, np.float32))
    router_w = np.ascontiguousarray(np.asarray(router_w, np.float32))
    w1 = np.asarray(w1, np.float32)
    b1 = np.asarray(b1, np.float32)
    w2 = np.asarray(w2, np.float32)
    b2 = np.asarray(b2, np.float32)

    B, S, _ = hidden_states.shape
    T = B * S
    x = hidden_states.reshape(T, H)

    top_i, rw = _route(x, router_w)

    sel_idx = []
    sel_gate = []
    for e in range(E):
        mask = top_i == e                                     # [T, K]
        rows = np.nonzero(mask.any(axis=1))[0]
        g = rw[rows[:, None], np.argmax(mask[rows], axis=1)[:, None]][:, 0]
        sel_idx.append(rows)
        sel_gate.append(g.astype(np.float32))

    # One job per (expert, token-chunk). Normally each expert fits in one
    # chunk and a single 8-core SPMD round runs everything; with an extreme
    # routing skew an expert's batch is split into <=C_MAX chunks (bounded
    # by SBUF) and extra rounds run.
    C_MAX = 2048
    jobs = []                                   # (expert, rows, gates)
    for e in range(E):
        rows, g = sel_idx[e], sel_gate[e]
        for off in range(0, max(len(rows), 1), C_MAX):
            jobs.append((e, rows[off:off + C_MAX], g[off:off + C_MAX]))

    n_rounds = -(-len(jobs) // N_CORES)
    cmax = max(len(r) for _, r, _ in jobs)
    C = max(P, -(-cmax // 8) * 8)

    if C not in _cache:
        _cache[C] = _build(C)
    nc = _cache[C]
    cbs = _blocks(C)

    w_bf = {}

    def expert_inputs(e):
        if e not in w_bf:
            # w1d[p, th*4096 + k*512 + fin] = w1[e][th*512+fin, k*128+p]
            # except the th0 region, which is m-major for contiguous
            # m-granular startup DMAs:
            # w1d[p, m*1024 + k*128 + fin] = w1[e][m*128+fin, k*128+p]
            w1r = (w1[e].reshape(NT, 512, KH, P).transpose(3, 0, 2, 1)
                   .reshape(P, NT * KH * 512)).copy()
            w1r[:, :KH * 512] = (w1[e][:512].reshape(4, P, KH, P)
                                 .transpose(3, 0, 2, 1).reshape(P, KH * 512))
            # w2d[p, n*4096 + m*128 + nin] = w2[e][n*128+nin, m*128+p]
            w2r = (w2[e].reshape(NT, P, MI, P).transpose(3, 0, 2, 1)
                   .reshape(P, NT * MI * P))
            w_bf[e] = {
                "w1d": np.ascontiguousarray(w1r).astype(ml_dtypes.bfloat16),
                "w2d": np.ascontiguousarray(w2r).astype(ml_dtypes.bfloat16),
                "b1d": np.ascontiguousarray(b1[e].reshape(MI, P).T),
            }
        return w_bf[e]

    global LAST_EXEC_TIME_NS
    LAST_EXEC_TIME_NS = 0
    out = np.zeros((T, H), np.float32)
    for r in range(n_rounds):
        batch = jobs[r * N_CORES:(r + 1) * N_CORES]
        while len(batch) < N_CORES:
            batch.append((0, sel_idx[0][:0], sel_gate[0][:0]))
        in_maps = []
        for e, rows, g in batch:
            n_e = len(rows)
            # xd block-major: [p, b(k cw)] with xd[p, ...] = x_token[k*128+p]
            xk = np.zeros((P, KH, C), np.float32)
            if n_e:
                xk[:, :, :n_e] = (x[rows].T.reshape(KH, P, n_e)
                                  .transpose(1, 0, 2))
            xd_e = np.concatenate(
                [np.ascontiguousarray(xk[:, :, coff:coff + cw])
                 .reshape(P, KH * cw) for coff, cw in cbs], axis=1)
            in_maps.append({
                "xd": xd_e.astype(ml_dtypes.bfloat16),
                **expert_inputs(e),
            })

        res = run_bass_kernel_spmd(nc, in_maps, list(range(N_CORES)), trace=TRACE)
        if res.exec_time_ns:
            LAST_EXEC_TIME_NS += res.exec_time_ns

        for core, (e, rows, g) in enumerate(batch):
            if len(rows):
                y = (res.results[core]["out"].reshape(P, NT, C)
                     .transpose(1, 0, 2).reshape(H, C))
                # row indices are unique within one job, so += is safe
                out[rows] += g[:, None] * (y[:, :len(rows)].T + b2[e][None, :])

    return out.reshape(B, S, H)
